# revision 15
# baseline (speedup 1.0000x reference)
"""Trainium2 Bass kernel for ragged segment-max + 1x1 conv + GeM pooling.

Problem: x [1,128,4096,16,11] f32 packed frames; seqL [32] ragged lengths;
W [256,128] 1x1-conv weight; p [4] GeM powers.  out [32, 256, 4] f32.

Strategy: shard whole segments across 8 cores (4 per core, LPT-balanced).
Per core: stream frames through a DVE max-reduce at 16-frame chunk
granularity (chunks segment-aligned via -1e30 padding; chunk data
transposed hw-major on host so the reduce inner dim is contiguous), then a
segmented max-scan over chunk maxes combines them into per-segment maxes
(reset gates at segment starts, uploaded as data so the program is uniform
across cores).  Segment results are gathered at data-driven chunk indices
via register-offset copies, pushed through the 1x1 conv on the PE, and the
GeM tail (clip, ln, *p, exp, mean, ^(1/p)) runs on ACT/DVE.
"""
import sys

import numpy as np

if "/opt/trn_rl_repo" not in sys.path:
    sys.path.insert(0, "/opt/trn_rl_repo")

# problem constants
B, S, C, O = 32, 4096, 128, 256
H, Wd = 16, 11
HW = H * Wd                  # 176
SPLIT = [4, 4, 4, 4]         # h split sizes
NPART = len(SPLIT)           # 4
WPP = HW // NPART            # 44 elems per GeM part
EPS = 1e-6
NCORES = 8
SEG_PER_CORE = B // NCORES   # 4

R = 16                       # frames per chunk (segment-alignment quantum)
CPB = 2                      # chunks per DMA buffer
BIG = 3.0e38
PAD = -1.0e30
RESCALE = 150.0              # GeM computed on t*RESCALE: ACT Ln table is only
                             # valid on ~[2^-64, 2^64], so keep (RESCALE*t)^p
                             # inside it for t in [EPS, ~50]
SMIN = 44.0 * 2.0 ** -60     # clamp sum(w2) so mean stays in the Ln window


_prog_cache = {}
DEBUG = False


def _plan(seqL):
    """Host planning: segment->core assignment + per-core chunk layout."""
    seqL = np.asarray(seqL).astype(np.int64).reshape(B)
    assert seqL.sum() == S and (seqL > 0).all()
    starts = np.concatenate([[0], np.cumsum(seqL)[:-1]])
    chunks = (seqL + R - 1) // R  # padded chunk count per segment

    # LPT: assign segments to cores balancing padded chunk totals, 4 per core
    order = np.argsort(-chunks, kind="stable")
    loads = [0] * NCORES
    members = [[] for _ in range(NCORES)]
    for sid in order:
        cand = sorted(range(NCORES), key=lambda c: (loads[c], c))
        for c in cand:
            if len(members[c]) < SEG_PER_CORE:
                members[c].append(int(sid))
                loads[c] += int(chunks[sid])
                break
    for c in range(NCORES):
        members[c].sort()

    ncch = [sum(int(chunks[s]) for s in members[c]) for c in range(NCORES)]
    NCH = max(ncch)
    NCH = ((NCH + CPB - 1) // CPB) * CPB  # multiple of chunks-per-buffer
    return {
        "seqL": seqL, "starts": starts, "chunks": chunks,
        "members": members, "NCH": NCH, "NB": NCH // CPB,
    }


def _repack_core(x_chw, plan, core):
    """Per-core DRAM stream [C, NCH*HW*R] (chunk-major; each chunk stored
    hw-major [hw, frame]), plus scan gate row and segment-end chunk ids."""
    NCH = plan["NCH"]
    members = plan["members"][core]
    out = np.full((C, NCH, HW, R), PAD, dtype=np.float32)
    gate_row = np.full((NCH,), BIG, dtype=np.float32)
    ends = np.zeros((SEG_PER_CORE,), dtype=np.int32)
    cpos = 0
    for j, sid in enumerate(members):
        L = int(plan["seqL"][sid]); s0 = int(plan["starts"][sid])
        k = int(plan["chunks"][sid])
        segp = np.full((C, k * R, HW), PAD, dtype=np.float32)
        segp[:, :L, :] = x_chw[:, s0:s0 + L, :]
        out[:, cpos:cpos + k] = segp.reshape(C, k, R, HW).transpose(0, 1, 3, 2)
        gate_row[cpos] = -BIG                           # reset at segment start
        cpos += k
        ends[j] = cpos - 1                              # last chunk of segment
    return out.reshape(C, NCH * HW * R), gate_row, ends


def _build_program(NCH, NB):
    import concourse.bass as bass
    import concourse.tile as tile
    from concourse import bacc, mybir

    F32 = mybir.dt.float32
    FREE_B = CPB * R * HW  # elems per partition per DMA buffer
    SH = SEG_PER_CORE * HW          # 704
    SN = SEG_PER_CORE * NPART       # 16

    nc = bacc.Bacc("TRN2", target_bir_lowering=False, debug=False,
                   num_devices=NCORES)
    x = nc.dram_tensor("x", [C, NCH * HW * R], F32, kind="ExternalInput")
    gate_d = nc.dram_tensor("gate", [C, HW * NCH], F32, kind="ExternalInput")
    gidx_d = nc.dram_tensor("gidx", [C, SEG_PER_CORE], mybir.dt.int32,
                            kind="ExternalInput")
    wt_d = nc.dram_tensor("wt", [C, O], F32, kind="ExternalInput")
    pvec_d = nc.dram_tensor("pvec", [C, SH], F32, kind="ExternalInput")
    qvec_d = nc.dram_tensor("qvec", [C, SN], F32, kind="ExternalInput")
    out_d = nc.dram_tensor("out", [C, 2 * SN], F32, kind="ExternalOutput")
    if DEBUG:
        dbg_d = nc.dram_tensor("dbg", [C, 5 * SH + 3 * SN], F32,
                               kind="ExternalOutput")

    with tile.TileContext(nc) as tc:
        with tc.tile_pool(name="xin", bufs=4) as xin, \
             tc.tile_pool(name="meta", bufs=1) as meta, \
             tc.tile_pool(name="work", bufs=1) as work, \
             tc.tile_pool(name="dram", bufs=1, space="DRAM") as dramp, \
             tc.tile_pool(name="psum", bufs=1, space="PSUM") as psum:
            cmax = work.tile([C, HW * NCH], F32, tag="cmax")
            gate = meta.tile([C, HW * NCH], F32, tag="gate")
            wt = meta.tile([C, O], F32, tag="wt")
            pvec = meta.tile([C, SH], F32, tag="pvec")
            qvec = meta.tile([C, SN], F32, tag="qvec")
            gidx = meta.tile([C, SEG_PER_CORE], mybir.dt.int32, tag="gidx")

            nc.sync.dma_start(gate[:], gate_d[:])
            nc.sync.dma_start(wt[:], wt_d[:])
            nc.sync.dma_start(pvec[:], pvec_d[:])
            nc.sync.dma_start(qvec[:], qvec_d[:])
            nc.sync.dma_start(gidx[:], gidx_d[:])

            # phase 1: stream buffers, per-chunk max-reduce
            for b in range(NB):
                t = xin.tile([C, FREE_B], F32, tag="xin")
                nc.sync.dma_start(t[:], x[:, b * FREE_B:(b + 1) * FREE_B])
                src = t[:].rearrange("p (c h r) -> p c h r", c=CPB, h=HW, r=R)
                dst = cmax[:].rearrange("p (h n) -> p h n", n=NCH)[
                    :, :, b * CPB:(b + 1) * CPB].rearrange("p h c -> p c h")
                nc.vector.reduce_max(dst, src, axis=mybir.AxisListType.X)

            # phase 2: segmented max-scan along chunk axis (resets via gate)
            sout = xin.tile([C, HW * NCH], F32, tag="xin")
            nc.vector.tensor_tensor_scan(
                sout[:], gate[:], cmax[:], initial=-BIG,
                op0=mybir.AluOpType.min, op1=mybir.AluOpType.max)

            # transpose scan output to chunk-major on ACT (idle engine),
            # round-trip through DRAM, then gather per-segment slices as an
            # indirect row-gather (row p*NCH + end_chunk of the [C*NCH, HW]
            # view; indices are host data so the program is core-uniform)
            soutT = xin.tile([C, HW * NCH], F32, tag="xin")
            nc.scalar.copy(
                soutT[:].rearrange("p (n h) -> p n h", h=HW),
                sout[:].rearrange("p (h n) -> p h n", n=NCH).rearrange(
                    "p h n -> p n h"))
            scratch = dramp.tile([C, NCH * HW], F32, tag="scratch")
            nc.sync.dma_start(scratch[:], soutT[:])
            table = scratch[:].rearrange("p (n h) -> (p n) h", h=HW)
            pooled = work.tile([C, SH], F32, tag="pooled")
            for j in range(SEG_PER_CORE):
                nc.gpsimd.indirect_dma_start(
                    out=pooled[:, j * HW:(j + 1) * HW],
                    out_offset=None,
                    in_=table,
                    in_offset=bass.IndirectOffsetOnAxis(
                        ap=gidx[:, j:j + 1], axis=0))

            # conv 1x1 (2 O-halves x 2 psum banks each) + GeM tail
            gtile = work.tile([C, 2 * SN], F32, tag="g")
            for half in range(2):
                t1 = work.tile([C, SH], F32, tag="t1x")
                for ns in range(2):
                    y = psum.tile([C, SH // 2], F32, tag=f"y{half}{ns}")
                    nc.tensor.matmul(
                        y[:],
                        wt[:, half * 128:(half + 1) * 128],
                        pooled[:, ns * (SH // 2):(ns + 1) * (SH // 2)],
                        start=True, stop=True)
                    nc.vector.tensor_scalar_max(
                        t1[:, ns * (SH // 2):(ns + 1) * (SH // 2)], y[:], EPS)
                u = work.tile([C, SH], F32, tag="ux")
                nc.scalar.activation(u[:], t1[:],
                                     mybir.ActivationFunctionType.Ln,
                                     scale=float(RESCALE))
                v = work.tile([C, SH], F32, tag="vx")
                nc.vector.tensor_mul(v[:], u[:], pvec[:])
                w2 = work.tile([C, SH], F32, tag="wx")
                nc.scalar.activation(w2[:], v[:],
                                     mybir.ActivationFunctionType.Exp)
                s = work.tile([C, SN], F32, tag="sx")
                nc.vector.reduce_sum(
                    s[:].rearrange("p (k one) -> p k one", one=1),
                    w2[:].rearrange("p (k m) -> p k m", m=WPP),
                    axis=mybir.AxisListType.X)
                nc.vector.tensor_scalar_max(s[:], s[:], float(SMIN))
                r2 = work.tile([C, SN], F32, tag="rx")
                nc.scalar.activation(r2[:], s[:],
                                     mybir.ActivationFunctionType.Ln,
                                     scale=float(1.0 / WPP))
                q2 = work.tile([C, SN], F32, tag="qx")
                nc.vector.tensor_mul(q2[:], r2[:], qvec[:])
                nc.vector.tensor_scalar_sub(q2[:], q2[:],
                                            float(np.log(RESCALE)))
                nc.scalar.activation(
                    gtile[:, half * SN:(half + 1) * SN],
                    q2[:], mybir.ActivationFunctionType.Exp)

            nc.sync.dma_start(out_d[:], gtile[:])
            if DEBUG:
                nc.sync.dma_start(dbg_d[:, 0:SH], pooled[:])
                nc.sync.dma_start(dbg_d[:, SH:2 * SH], t1[:])
                nc.sync.dma_start(dbg_d[:, 2 * SH:3 * SH], u[:])
                nc.sync.dma_start(dbg_d[:, 3 * SH:4 * SH], v[:])
                nc.sync.dma_start(dbg_d[:, 4 * SH:5 * SH], w2[:])
                nc.sync.dma_start(dbg_d[:, 5 * SH:5 * SH + SN], s[:])
                nc.sync.dma_start(dbg_d[:, 5 * SH + SN:5 * SH + 2 * SN], r2[:])
                nc.sync.dma_start(dbg_d[:, 5 * SH + 2 * SN:], q2[:])
    nc.compile()
    return nc


def _run_device(nc, in_maps):
    from concourse.bass_utils import run_bass_kernel_spmd
    res = run_bass_kernel_spmd(nc, in_maps, list(range(NCORES)))
    return res.results


def _make_in_maps(x, plan, W, p):
    x_chw = np.ascontiguousarray(x[0]).reshape(C, S, HW)
    NCH = plan["NCH"]
    wt = np.ascontiguousarray(W.T).astype(np.float32)               # [C, O]
    prow = np.repeat(p.astype(np.float32), WPP)                     # [HW]
    pvec = np.ascontiguousarray(
        np.broadcast_to(np.tile(prow, SEG_PER_CORE)[None, :],
                        (C, SEG_PER_CORE * HW))).astype(np.float32)
    qrow = (1.0 / p.astype(np.float32))                             # [NPART]
    qvec = np.ascontiguousarray(
        np.broadcast_to(np.tile(qrow, SEG_PER_CORE)[None, :],
                        (C, SEG_PER_CORE * NPART))).astype(np.float32)
    in_maps = []
    for core in range(NCORES):
        xc, gate_row, ends = _repack_core(x_chw, plan, core)
        gate = np.ascontiguousarray(
            np.broadcast_to(
                np.repeat(gate_row[None, :], HW, axis=0).reshape(-1)[None, :],
                (C, HW * NCH))).astype(np.float32)
        in_maps.append({
            "x": xc, "gate": gate,
            "gidx": np.ascontiguousarray(
                (np.arange(C, dtype=np.int32)[:, None] * np.int32(NCH))
                + ends[None, :].astype(np.int32)),
            "wt": wt, "pvec": pvec, "qvec": qvec,
        })
    return in_maps


def kernel(x, seqL, W, p):
    x = np.asarray(x, dtype=np.float32)
    W = np.asarray(W, dtype=np.float32)
    p = np.asarray(p, dtype=np.float32)
    plan = _plan(seqL)

    in_maps = _make_in_maps(x, plan, W, p)

    key = (plan["NCH"], plan["NB"])
    if key not in _prog_cache:
        _prog_cache[key] = _build_program(plan["NCH"], plan["NB"])
    nc = _prog_cache[key]

    results = _run_device(nc, in_maps)

    SN = SEG_PER_CORE * NPART
    out = np.zeros((B, O, NPART), dtype=np.float32)
    for core in range(NCORES):
        g = results[core]["out"]  # [C, 2*SN]
        for j, sid in enumerate(plan["members"][core]):
            for half in range(2):
                blk = g[:, half * SN + j * NPART: half * SN + (j + 1) * NPART]
                out[sid, half * 128:(half + 1) * 128, :] = blk
    return out
